# revision 1
# baseline (speedup 1.0000x reference)
"""CrossModalAttention Trainium2 kernel (8-core data parallel).

Math: with seq_len=1, softmax over one key == 1, so each MultiheadAttention
collapses to   att = (kv @ Wv.T + bv) @ Wo.T + bo = kv @ Wc.T + bc
with Wc = Wo @ Wv (256x256) and bc = bv @ Wo.T + bo, followed by
    out = LayerNorm(x + att) * g + b.

Device dataflow (per core, 16384 rows per modality):
  - Host passes activations TRANSPOSED (feat on partitions, fp32r-rounded)
    so the 256x256 weight is the PE-stationary operand and the activation
    streams as the moving operand at full float32r rate (n=512).
  - att.T accumulates in PSUM; a DVE pass adds the residual x.T (+ per-
    partition bias bc) producing z.T in SBUF.
  - PE transposes z back to natural layout (rows on partitions) into PSUM.
  - batched bn_stats/bn_aggr give per-row mean/var; ACT applies (z-m)*rstd.
  - Optional (non-trivial g/b only): elementwise g,b application.
"""

import os
import numpy as np

N_CORES = 8
B = 131072
E = 256
EPS = 1e-5
ROWS = B // N_CORES          # rows per core per modality
SUPER = 1024                 # rows per DMA super-tile (2 MB for both modalities)
SUB = 512                    # rows per compute unit (matmul moving dim)
N_SUPER = ROWS // SUPER
N_SUB = SUPER // SUB
RT = SUB // 128              # row-tiles per unit

_PROGRAM_CACHE = {}


def _build_program(generic_gb, generic_bc):
    import concourse.bass as bass
    import concourse.tile as tile
    from concourse import bacc, mybir
    from concourse.masks import make_identity

    f32 = mybir.dt.float32
    f32r = mybir.dt.float32r
    AF = mybir.ActivationFunctionType
    OP = mybir.AluOpType

    nc = bacc.Bacc("TRN2")

    # ---- DRAM I/O ----
    # xT[mod] = per-core shard transposed: (2, E, ROWS), fp32r-rounded.
    xT = nc.dram_tensor("xT", [2, E, ROWS], f32r, kind="ExternalInput")
    # w[mod] = Wc[mod].T laid out (feat_in, feat_out); mod 0 produces the
    # audio output (source = text), mod 1 the text output (source = audio).
    w = nc.dram_tensor("w", [2, E, E], f32r, kind="ExternalInput")
    bc = nc.dram_tensor("bc", [2, E, 1], f32, kind="ExternalInput")
    if generic_gb:
        g = nc.dram_tensor("g", [2, 1, E], f32, kind="ExternalInput")
        b = nc.dram_tensor("b", [2, 1, E], f32, kind="ExternalInput")
    y = nc.dram_tensor("y", [2, ROWS, E], f32, kind="ExternalOutput")

    # DRAM views
    xT_v = xT.rearrange("m (c p) n -> p m c n", p=128)
    w_v = w.rearrange("m (k p) (mm q) -> p m k mm q", p=128, q=128)
    bc_v = bc.rearrange("m (c p) one -> p m (c one)", p=128)
    y_v = y.rearrange("m (t p) d -> p m t d", p=128)

    with tile.TileContext(nc) as tc:
        with (
            tc.tile_pool(name="const", bufs=1) as const_pool,
            tc.tile_pool(name="xin", bufs=5) as xin_pool,
            tc.tile_pool(name="yout", bufs=5) as yout_pool,
            tc.tile_pool(name="zt", bufs=3) as zt_pool,
            tc.tile_pool(name="stats", bufs=8) as stats_pool,
            tc.tile_pool(name="attps", bufs=2, space="PSUM") as attps_pool,
            tc.tile_pool(name="znps", bufs=4, space="PSUM") as znps_pool,
        ):
            # ---- constants ----
            w_sb = const_pool.tile([128, 2, 2, 2, 128], f32r)  # [p, mod, k, m, q]
            nc.sync.dma_start(out=w_sb, in_=w_v)
            bc_sb = const_pool.tile([128, 2, 2], f32)  # [p, mod, chunk]
            nc.sync.dma_start(out=bc_sb, in_=bc_v)
            ident = const_pool.tile([128, 128], f32)
            make_identity(nc, ident)
            eps_sb = const_pool.tile([128, 1], f32)
            nc.vector.memset(eps_sb, EPS)
            if generic_gb:
                gb_sb = const_pool.tile([128, 2, 2, E], f32)  # [p, mod, (g,b), E]
                for mod in range(2):
                    nc.sync.dma_start(
                        out=gb_sb[:, mod, 0], in_=g[mod].to_broadcast((128, E))
                    )
                    nc.sync.dma_start(
                        out=gb_sb[:, mod, 1], in_=b[mod].to_broadcast((128, E))
                    )

            for sp in range(N_SUPER):
                # ---- load super-tile (one 4 MB DMA, both modalities) ----
                xT_sb = xin_pool.tile([128, 2, 2, SUPER], f32r, tag="xin")
                nc.sync.dma_start(
                    out=xT_sb,
                    in_=xT_v[:, :, :, sp * SUPER:(sp + 1) * SUPER],
                )
                xT_f = xT_sb.bitcast(f32)
                y_sb = yout_pool.tile([128, 2, SUPER // 128, E], f32, tag="yout")

                for sub in range(N_SUB):
                    r0 = sub * SUB
                    for mod in range(2):
                        # source modality for attention is the OTHER stream
                        src = 1 - mod
                        # ---- matmul: att.T[m] += W[k,m].T @ xT[k], n=512 ----
                        att_ps = attps_pool.tile([128, 2, SUB], f32, tag="att")
                        for m in range(2):
                            for k in range(2):
                                nc.tensor.matmul(
                                    att_ps[:, m, :],
                                    w_sb[:, mod, k, m, :],
                                    xT_sb[:, src, k, r0:r0 + SUB],
                                    start=(k == 0),
                                    stop=(k == 1),
                                    skip_group_check=True,
                                )
                        # ---- z.T = att.T + bc + x.T (residual) ----
                        zT_sb = zt_pool.tile([128, 2, SUB], f32, tag="zt")
                        if generic_bc:
                            for c in range(2):
                                nc.vector.scalar_tensor_tensor(
                                    out=zT_sb[:, c, :],
                                    in0=att_ps[:, c, :],
                                    scalar=bc_sb[:, mod, c:c + 1],
                                    in1=xT_f[:, mod, c, r0:r0 + SUB],
                                    op0=OP.add,
                                    op1=OP.add,
                                )
                        else:
                            nc.vector.tensor_add(
                                zT_sb,
                                att_ps,
                                xT_f[:, mod, :, r0:r0 + SUB],
                            )
                        # ---- transposes + stats in 256-row half-units ----
                        zn_tiles = []
                        st = stats_pool.tile([128, RT, 6], f32, tag="st")
                        for h in range(RT // 2):
                            # transpose z back to natural layout;
                            # zn_ps[:, rt, :] holds rows of tile 2h+rt
                            zn_ps = znps_pool.tile([128, 2, 256], f32, tag="zn")
                            zn_tiles.append(zn_ps)
                            for rt in range(2):
                                for c in range(2):
                                    nc.tensor.matmul(
                                        zn_ps[:, rt, c * 128:(c + 1) * 128],
                                        zT_sb[:, c,
                                              (2 * h + rt) * 128:
                                              (2 * h + rt + 1) * 128],
                                        ident,
                                        is_transpose=True,
                                        start=(rt == 0 and c == 0),
                                        stop=(rt == 1 and c == 1),
                                        skip_group_check=True,
                                    )
                            # layernorm stats (one group per call: HW limit)
                            for rt in range(2):
                                nc.vector.bn_stats(
                                    out=st[:, 2 * h + rt, :],
                                    in_=zn_ps[:, rt, :],
                                )
                        # aggregate + batched per-unit scalar math ([128, RT])
                        mv = stats_pool.tile([128, RT, 2], f32, tag="mv")
                        for k in range(RT):
                            nc.vector.bn_aggr(out=mv[:, k, :], in_=st[:, k, :])
                        sd = stats_pool.tile([128, RT], f32, tag="sd")
                        nc.scalar.activation(
                            out=sd, in_=mv[:, :, 1], func=AF.Sqrt,
                            bias=eps_sb, scale=1.0,
                        )
                        rstd = stats_pool.tile([128, RT], f32, tag="rstd")
                        nc.vector.reciprocal(out=rstd, in_=sd)
                        nmrs = stats_pool.tile([128, RT], f32, tag="nmrs")
                        nc.vector.scalar_tensor_tensor(
                            out=nmrs, in0=mv[:, :, 0], scalar=-1.0,
                            in1=rstd, op0=OP.mult, op1=OP.mult,
                        )
                        # normalize: y = (z - m) * rstd
                        ti = sub * RT
                        for rt in range(RT):
                            nc.scalar.activation(
                                out=y_sb[:, mod, ti + rt, :],
                                in_=zn_tiles[rt // 2][:, rt % 2, :],
                                func=AF.Identity,
                                bias=nmrs[:, rt:rt + 1],
                                scale=rstd[:, rt:rt + 1],
                            )
                            if generic_gb:
                                nc.vector.tensor_mul(
                                    y_sb[:, mod, ti + rt, :],
                                    y_sb[:, mod, ti + rt, :],
                                    gb_sb[:, mod, 0],
                                )
                                nc.vector.tensor_add(
                                    y_sb[:, mod, ti + rt, :],
                                    y_sb[:, mod, ti + rt, :],
                                    gb_sb[:, mod, 1],
                                )

                # ---- store super-tile (2 MB DMA per modality) ----
                t0 = sp * (SUPER // 128)
                for mod in range(2):
                    nc.sync.dma_start(
                        out=y_v[:, mod, t0:t0 + SUPER // 128, :],
                        in_=y_sb[:, mod],
                    )

    nc.finalize()
    return nc


def _get_program(generic_gb, generic_bc):
    key = (bool(generic_gb), bool(generic_bc))
    if key not in _PROGRAM_CACHE:
        _PROGRAM_CACHE[key] = _build_program(*key)
    return _PROGRAM_CACHE[key]


def _prep_host(audio_embed, text_embed,
               a2t_in_w, a2t_in_b, a2t_out_w, a2t_out_b,
               t2a_in_w, t2a_in_b, t2a_out_w, t2a_out_b,
               ln1_g, ln1_b, ln2_g, ln2_b):
    f = np.float32
    # fold the two projections: att = kv @ (Wo @ Wv).T + (bv @ Wo.T + bo)
    wv_a, bv_a = a2t_in_w[2 * E:], a2t_in_b[2 * E:]
    wv_t, bv_t = t2a_in_w[2 * E:], t2a_in_b[2 * E:]
    wc_a = (a2t_out_w.astype(np.float64) @ wv_a.astype(np.float64)).astype(f)
    wc_t = (t2a_out_w.astype(np.float64) @ wv_t.astype(np.float64)).astype(f)
    bc_a = (bv_a.astype(np.float64) @ a2t_out_w.T.astype(np.float64)
            + a2t_out_b.astype(np.float64)).astype(f)
    bc_t = (bv_t.astype(np.float64) @ t2a_out_w.T.astype(np.float64)
            + t2a_out_b.astype(np.float64)).astype(f)

    generic_gb = not (
        np.all(ln1_g == 1.0) and np.all(ln1_b == 0.0)
        and np.all(ln2_g == 1.0) and np.all(ln2_b == 0.0)
    )
    generic_bc = not (np.all(bc_a == 0.0) and np.all(bc_t == 0.0))

    audio = np.ascontiguousarray(audio_embed, dtype=f)
    text = np.ascontiguousarray(text_embed, dtype=f)

    from concurrent.futures import ThreadPoolExecutor
    from neuron_dtypes._impl import fp32r as _fp32r

    def round_f32r(x):
        # round-to-nearest into the fp32r (11-bit mantissa) grid, keeping
        # the float32 container — what the PE expects for fp32r operands
        u = np.ascontiguousarray(x, dtype=f).reshape(-1).view(np.uint32)
        return _fp32r.cast_fp32_to_fp32r(u.size, u).view(f).reshape(x.shape)

    def shard_xT(c):
        out = np.empty((2, E, ROWS), f)
        out[0] = audio[c * ROWS:(c + 1) * ROWS].T
        out[1] = text[c * ROWS:(c + 1) * ROWS].T
        return round_f32r(out)

    with ThreadPoolExecutor(max_workers=8) as ex:
        xTs = list(ex.map(shard_xT, range(N_CORES)))

    w_all = round_f32r(np.stack([wc_a.T, wc_t.T]))
    bc_all = np.stack([bc_a.reshape(E, 1), bc_t.reshape(E, 1)])
    in_maps = []
    for c in range(N_CORES):
        m = {"xT": xTs[c], "w": w_all, "bc": bc_all}
        if generic_gb:
            m["g"] = np.stack([
                np.ascontiguousarray(ln1_g, dtype=f).reshape(1, E),
                np.ascontiguousarray(ln2_g, dtype=f).reshape(1, E),
            ])
            m["b"] = np.stack([
                np.ascontiguousarray(ln1_b, dtype=f).reshape(1, E),
                np.ascontiguousarray(ln2_b, dtype=f).reshape(1, E),
            ])
        in_maps.append(m)
    return in_maps, generic_gb, generic_bc


def _run(in_maps, generic_gb, generic_bc, trace=False):
    import sys
    if "/opt/trn_rl_repo" not in sys.path:
        sys.path.insert(0, "/opt/trn_rl_repo")
    from concourse.bass_utils import run_bass_kernel_spmd

    nc = _get_program(generic_gb, generic_bc)
    res = run_bass_kernel_spmd(
        nc, in_maps, list(range(N_CORES)), trace=trace,
    )
    return res


def kernel(**inputs):
    import sys
    if "/opt/trn_rl_repo" not in sys.path:
        sys.path.insert(0, "/opt/trn_rl_repo")
    in_maps, generic_gb, generic_bc = _prep_host(**inputs)
    res = _run(in_maps, generic_gb, generic_bc,
               trace=bool(os.environ.get("KERNEL_TRACE")))
    audio_out = np.concatenate([r["y"][0] for r in res.results], axis=0)
    text_out = np.concatenate([r["y"][1] for r in res.results], axis=0)
    kernel.last_exec_time_ns = res.exec_time_ns
    kernel.last_results = res
    return (audio_out, text_out)

